# revision 1
# baseline (speedup 1.0000x reference)
"""DNC scatter_memory kernel for 8 Trainium2 NeuronCores.

Strategy: the only heavy tensor is L [8192,8192] (256MB f32). The
returned y depends on L only through Q = M_new^T @ W_read_new [64,4]
(read_v_new = Q^T), and every L-term of Q is a ROW-contraction of L:

  W_fwd-side:  M^T(s*F1) = (M*s)^T L r,  M^T F2 = (M^T L)(w*r)
  W_bwd-side:  M^T B1, (M*w)^T B2 with B1 = L^T(s*r), B2 = L^T r
  (rank-1 / diagonal fixup terms are tiny host-side products)

So the device computes a single fused product
  U = S^T @ L   with  S = [s*r, r, M*s, M]  [8192, 136]
row-sharded over 8 cores (U partials summed on host), and the host
assembles Q and y. L is read exactly ONCE per element, as fp8_e4m3
(scaled 2^13; ~1e-4 end-to-end error vs the 2e-2 budget, since every
output is an average of 8192 independently-rounded terms). S is fp8
with per-column-group power-of-2 scales chosen at runtime; U partials
return as fp8_e4m3, scaled per row on-device (DVE tensor_scalar_mul)
with host-computed true-bound power-of-2 scales so the cast can never
overflow.

Per core: the 1024-row shard of L*2^13 (8MB fp8) is pre-swizzled on the
host into 16 contiguous 512KB "slabs" [128 part, 8 row-tiles x 512
cols]. One slab x one stationary-half = one PSUM-bank accumulation
group of 4 fp8-DoubleRow matmuls (256-row contraction each, operand
layout [p, 2, free] as in concourse's tile_matmul fp8 path). Slabs
rotate through an SBUF ring and the 8 PSUM banks (even banks: 72-col
half [s*r, r, M*s]; odd banks: 64-col half [M]) giving full DMA/PE/DVE
overlap; DVE downcasts bank copies to bf16; outputs drain in 4 chunks
on the sync HWDGE queue behind the inputs.

Host side (LSTM controller, interface partition, usage/allocation sort,
content softmaxes, M update, Q assembly, final projection) is tiny.
"""
import numpy as np
import ml_dtypes

N, Wd, R, OUT, IN = 8192, 64, 4, 256, 256
NCORES = 8
SH = N // NCORES          # 1024 rows per core
RT = SH // 128            # 8 row tiles per shard
NSLAB = N // 512          # 16 slabs: one per 512-col chunk
NB = 16                   # slab ring depth (SBUF slots of 4KB/partition)
STK = 144                 # stack row stride (136 cols + pad; %16 for DR)
H0, H1 = 72, 64           # stationary halves: [s*r,r,M*s] and [M]
SCALE = 8192.0            # L quantization scale (2^13)

F8 = ml_dtypes.float8_e4m3
BF16 = ml_dtypes.bfloat16

_cached = {}


def _sig(x):
    return 1.0 / (1.0 + np.exp(-x))


def _softplus(x):
    return np.log1p(np.exp(-np.abs(x))) + np.maximum(x, 0.0)


def _softmax0(x):
    m = x.max(axis=0, keepdims=True)
    e = np.exp(x - m)
    return e / e.sum(axis=0, keepdims=True)


def _l2norm(a, axis):
    return a / np.sqrt(np.maximum((a * a).sum(axis=axis, keepdims=True), 1e-12))


def _host_pre(x, dense_k, dense_b, lstm_k, lstm_r, lstm_b, W_output, W_interface,
              W_read_out, h, c, M, usage, L, W_prec, W_read, W_write, read_v):
    f32 = np.float32
    xd = x.reshape(1, -1) @ dense_k + dense_b
    seq = np.concatenate([xd, read_v], axis=0)
    for t in range(seq.shape[0]):
        z = seq[t][None, :] @ lstm_k + h @ lstm_r + lstm_b
        zi, zf, zg, zo = np.split(z, 4, axis=1)
        c = _sig(zf) * c + _sig(zi) * np.tanh(zg)
        h = _sig(zo) * np.tanh(c)
    out_v = h @ W_output
    iface = (h @ W_interface)[0]

    sizes = [R * Wd, R, Wd, 1, Wd, Wd, R, 1, 1, 3 * R]
    offs = np.cumsum([0] + sizes)
    p = [iface[offs[i]:offs[i + 1]] for i in range(10)]
    k_read = p[0].reshape(R, Wd)
    b_read = 1.0 + _softplus(p[1])[None, :]
    k_write = p[2][None, :]
    b_write = 1.0 + _softplus(p[3])[None, :]
    erase = _sig(p[4])[None, :]
    write_v = p[5][None, :]
    free_gates = _sig(p[6])[None, :]
    alloc_gate = _sig(p[7][0])
    write_gate = _sig(p[8][0])
    read_modes = _softmax0(p[9].reshape(3, R))

    retention = np.prod(1.0 - free_gates * W_read, axis=1, keepdims=True).astype(f32)
    usage = ((usage + W_write - usage * W_write) * retention).astype(f32)
    u_flat = usage[:, 0]
    order = np.argsort(u_flat, kind="stable")
    sorted_u = u_flat[order]
    cp = np.cumprod(sorted_u).astype(f32)
    cp_excl = np.concatenate([np.ones((1,), f32), cp[:-1]])
    unorder = (1.0 - sorted_u) * cp_excl
    W_alloc = np.zeros((N,), f32)
    W_alloc[order] = unorder
    W_alloc = W_alloc[:, None]

    nM = _l2norm(M, 1)
    W_lookup_w = _softmax0(nM @ _l2norm(k_write, 1).T * b_write)
    W_write_new = (write_gate * (alloc_gate * W_alloc
                                 + (1.0 - alloc_gate) * W_lookup_w)).astype(f32)
    M_new = (M * (1.0 - W_write_new @ erase) + W_write_new @ write_v).astype(f32)

    w = W_write_new[:, 0]
    s = (1.0 - w).astype(f32)
    r = W_read.astype(f32)
    return dict(out_v=out_v.astype(f32), k_read=k_read, b_read=b_read,
                read_modes=read_modes, M_new=M_new, w=w, s=s, r=r,
                W_read_out=W_read_out, W_prec=W_prec)


def _make_stack(pr):
    """[N, 136] f32 stack S = [s*r, r, M*s, M] and per-column scales."""
    w, s, r, M = pr["w"], pr["s"], pr["r"], pr["M_new"]
    stack = np.concatenate(
        [s[:, None] * r, r, M * s[:, None], M], axis=1).astype(np.float32)
    sy = np.empty(136, np.float32)
    for cols in (slice(0, 8), slice(8, 136)):
        amax = float(np.abs(stack[:, cols]).max())
        e = np.clip(np.floor(np.log2(160.0 / max(amax, 1e-30))), -40.0, 40.0)
        sy[cols] = 2.0 ** e
    return stack, sy


def _host_post_q(pr, U, diagL):
    """Assemble y from U = S^T @ L [136, N] (f32, unscaled)."""
    w, s, r = pr["w"], pr["s"], pr["r"]
    p = pr["W_prec"][:, 0]
    M = pr["M_new"]
    Ld_new = ((1.0 - 2.0 * w) * diagL + w * p).astype(np.float32)

    B1 = U[0:4].T                       # L^T (s*r)   [N,4]
    B2 = U[4:8].T                       # L^T r       [N,4]
    G1 = U[8:72] @ r                    # (M*s)^T L r        [64,4]
    G2 = U[72:136] @ (w[:, None] * r)   # (M^T L)(w*r)       [64,4]

    pr_r = p @ r
    wr_r = w @ r
    Mld_r = (M * Ld_new[:, None]).T @ r
    Q_fwd = G1 - G2 + np.outer(M.T @ w, pr_r) - Mld_r
    Q_bwd = (M.T @ B1) - ((M * w[:, None]).T @ B2) \
        + np.outer(M.T @ p, wr_r) - Mld_r

    W_lookup_r = _softmax0(_l2norm(M, 1) @ _l2norm(pr["k_read"], 1).T
                           * pr["b_read"])
    Q_look = M.T @ W_lookup_r
    modes = pr["read_modes"]
    Q = modes[0][None, :] * Q_bwd + modes[1][None, :] * Q_look \
        + modes[2][None, :] * Q_fwd                     # [64, 4]

    read_v_new = Q.T                                    # [4, 64]
    y = pr["out_v"] + read_v_new.reshape(1, R * Wd) @ pr["W_read_out"]
    return y.astype(np.float32)


def _build_bass():
    """Raw-Bass SPMD kernel: U = S^T @ L, single row-layout fp8 read.

    Inputs per core (host pre-swizzled):
      X  [NSLAB*128, 4096] fp8: slab s at rows [128s : 128s+128], SBUF
         partition p byte 512t+c = shard element [row 128t+p, col
         512s+c] of the L row shard.
      S  [128, RT*STK] fp8: partition p, element STK*t+j = stack column
         j of shard row 128t+p (j<72: [s*r, r, M*s]; 80<=j<144: [M];
         72:80 zero pad so both stationary bases are 16B-aligned).
    One (slab, half) = one PSUM accumulation group of 4 fp8 DoubleRow
    matmuls (start->stop full bank lifecycle); even banks take the
    72-col half, odd banks the 64-col half. DVE downcasts each bank to
    fp8 into the shared obx staging buffer (h1's 64 rows under h0's
    72); outputs drain in 4 merged strided chunks on the sync queue after
    all input dma_starts (SEQ-blocking waits can't stall queued input
    transfers). The last slab ships as two half-DMAs (fresh sems;
    per-engine FIFO makes each threshold imply its half) to shorten the
    tail.
    """
    import concourse.bass as bass
    import concourse.mybir as mybir
    from contextlib import ExitStack

    f32 = mybir.dt.float32
    bf16 = mybir.dt.bfloat16
    f8 = mybir.dt.float8e4
    nc = bass.Bass()
    X = nc.dram_tensor("X", [NSLAB * 128, 4096], f8, kind="ExternalInput")
    S = nc.dram_tensor("S", [128, RT * STK], f8, kind="ExternalInput")
    SC = nc.dram_tensor("SC", [128, 2], f32, kind="ExternalInput")
    # single output: h0 rows 0:72 in cols [0:N), h1 rows 0:64 in [N:2N)
    # (rows 64:72 of the h1 region are never written; host ignores them)
    U = nc.dram_tensor("U", [H0, 2 * N], f8, kind="ExternalOutput")

    with ExitStack() as es:
        xb = es.enter_context(nc.sbuf_tensor("xb", [128, NB * 4096], f8))
        sb = es.enter_context(nc.sbuf_tensor("sb", [128, RT * STK], f8))
        scb = es.enter_context(nc.sbuf_tensor("scb", [128, 2], f32))
        obx = es.enter_context(nc.sbuf_tensor("obx", [H0, 2 * NSLAB * 512], f8))
        ps = [es.enter_context(
            nc.psum_tensor(f"ps{b}", [H0 if b % 2 == 0 else H1, 512], f32))
            for b in range(8)]
        sem_s = es.enter_context(nc.semaphore(name="sem_s"))
        slot = [es.enter_context(nc.semaphore(name=f"slot{i}"))
                for i in range(NB)]
        sem_q = [[es.enter_context(nc.semaphore(name=f"sem_q{j}_{i}"))
                  for i in range(RT // 2)] for j in range(2)]
        pe_sem = es.enter_context(nc.semaphore(name="pe_sem"))
        dve_e = es.enter_context(nc.semaphore(name="dve_e"))
        dve_o = es.enter_context(nc.semaphore(name="dve_o"))
        ms_sem = es.enter_context(nc.semaphore(name="ms_sem"))
        out_sem = es.enter_context(nc.semaphore(name="out_sem"))
        blk = es.enter_context(nc.Block())

        @blk.scalar
        def _(scalar):
            # tiny stack + scale loads on the ACT HWDGE queue
            scalar.dma_start(out=sb[:, :], in_=S[:, :]).then_inc(sem_s, 16)
            scalar.dma_start(out=scb[:, :], in_=SC[:, :]).then_inc(sem_s, 16)
            # ACT drains the odd banks (h1 groups) in parallel with DVE —
            # ScalarE and VectorE may touch PSUM concurrently on
            # different banks. out = psum * scale, cast to fp8.
            scalar.wait_ge(sem_s, 32)   # scb resident (DMA is async)
            for go in range(NSLAB):
                g = 2 * go + 1
                scalar.wait_ge(pe_sem, g + 1)
                scalar.mul(obx[:H1, NSLAB * 512 + 512 * go:
                               NSLAB * 512 + 512 * (go + 1)],
                           ps[g % 8][:, :],
                           scb[:H1, 1:2]).then_inc(dve_o, 1)

        @blk.sync
        def _(sync):
            for s in range(NSLAB):
                if s >= NB:   # ring reuse: matmuls of slab s-NB done
                    sync.wait_ge(pe_sem, 2 * (s - NB) + 2)
                k = s % NB
                if s >= NSLAB - 2:
                    # quarter-DMAs (one per DoubleRow matmul's stripe) so
                    # only the final matmul pair trails the last transfer
                    for qq in range(RT // 2):
                        sync.dma_start(
                            out=xb[:, 4096 * k + 1024 * qq:
                                   4096 * k + 1024 * (qq + 1)],
                            in_=X[128 * s:128 * (s + 1),
                                  1024 * qq:1024 * (qq + 1)],
                        ).then_inc(sem_q[s - (NSLAB - 2)][qq], 16)
                    continue
                sync.dma_start(
                    out=xb[:, 4096 * k:4096 * (k + 1)],
                    in_=X[128 * s:128 * (s + 1), :],
                ).then_inc(slot[k], 16)
            # outputs behind all queued inputs; chunks get finer toward
            # the end so the last transfers are small and fire early.
            # Both halves ship in ONE strided DMA per chunk (single
            # HWDGE descriptor-gen on the critical tail).
            Ur = U.rearrange("p (r c) -> p r c", r=2)
            obr = obx.rearrange("p (r c) -> p r c", r=2)
            sync.wait_ge(ms_sem, 1)
            for s0, s1 in ((0, 8), (8, 14), (14, 15), (15, 16)):
                sync.wait_ge(dve_e, s1)
                sync.wait_ge(dve_o, s1)
                sync.dma_start(
                    out=Ur[:, :, 512 * s0:512 * s1],
                    in_=obr[:, :, 512 * s0:512 * s1],
                ).then_inc(out_sem, 16)

        @blk.tensor
        def _(tensor):
            tensor.wait_ge(sem_s, 32)
            sbr = sb.rearrange("p (t j) -> p t j", j=STK)
            xbr = xb.rearrange("p (q c) -> p q c", c=512)
            halves = ((0, H0), (80, H1))

            def mk_mm(s, k, h, u):
                j0, hh = halves[h]
                b = (2 * s + h) % 8
                mm = tensor.matmul(
                    ps[b][:, :],
                    sbr[:, 2 * u:2 * u + 2, j0:j0 + hh],
                    xbr[:, 8 * k + 2 * u:8 * k + 2 * u + 2, :],
                    start=(u == 0), stop=(u == RT // 2 - 1),
                    perf_mode=mybir.MatmulPerfMode.DoubleRow,
                )
                if u == RT // 2 - 1:
                    mm.then_inc(pe_sem, 1)

            for s in range(NSLAB):
                k = s % NB
                if s < NSLAB - 2:
                    tensor.wait_ge(slot[k], 16 * (s // NB + 1))
                    for h in range(2):
                        g = 2 * s + h
                        if g >= 8:   # bank reuse: copier drained g-8
                            tensor.wait_ge(dve_e if h == 0 else dve_o,
                                           g // 2 - 4 + 1)
                        for u in range(RT // 2):
                            mk_mm(s, k, h, u)
                else:
                    # quarters arrive progressively; interleave halves per
                    # u so only the final matmul pair trails the transfer
                    tensor.wait_ge(dve_e, s - 3)
                    tensor.wait_ge(dve_o, s - 3)
                    for u in range(RT // 2):
                        tensor.wait_ge(sem_q[s - (NSLAB - 2)][u], 16)
                        for h in range(2):
                            mk_mm(s, k, h, u)

        @blk.gpsimd
        def _(gpsimd):
            # zero the never-written rows 64:72 of the h1 region so the
            # merged out-DMAs ship defined bytes (runs during the lead)
            gpsimd.memset(obx[H1:H0, NSLAB * 512:2 * NSLAB * 512],
                          0).then_inc(ms_sem, 1)

        @blk.vector
        def _(vector):
            vector.wait_ge(sem_s, 32)
            for ge in range(NSLAB):     # even banks (h0 groups) on DVE
                g = 2 * ge
                vector.wait_ge(pe_sem, g + 1)
                vector.tensor_scalar_mul(
                    obx[:, 512 * ge:512 * (ge + 1)],
                    ps[g % 8][:, :],
                    scb[:H0, 0:1],
                ).then_inc(dve_e, 1)

    return nc


def _get_bass():
    if "nc" not in _cached:
        _cached["nc"] = _build_bass()
    return _cached["nc"]


def _swizzle_shard(S8):
    """[1024, 8192] fp8 shard -> [16*128, 4096] slab-major layout."""
    # want A[cc, p, t, c] = S8[128t + p, 512cc + c]
    a = S8.reshape(RT, 128, NSLAB, 512).transpose(2, 1, 0, 3)
    return np.ascontiguousarray(a).reshape(NSLAB * 128, RT * 512)


def _prep_core_inputs(L, stack, sy):
    """Quantize L (scaled) + stack to fp8, build per-core input maps.

    Also computes per-U-row power-of-2 output scales from the TRUE bound
    |U[j,:]| <= sum_i |S_q[i,j]| * max|L_q| (x1.14 quantization margin),
    so the on-device fp8 cast of scaled PSUM rows can never overflow.
    """
    L8 = (L * np.float32(SCALE)).astype(F8)
    scaled_f = stack * sy[None, :]
    scaled = scaled_f.astype(F8)
    st8 = np.zeros((N, STK), F8)
    st8[:, :H0] = scaled[:, :H0]
    st8[:, 80:80 + H1] = scaled[:, H0:136]

    lmax = float(np.abs(L).max()) * SCALE
    bound = np.abs(scaled_f).sum(axis=0) * lmax * 1.14          # [136]
    e = np.clip(np.floor(np.log2(240.0 / np.maximum(bound, 1e-30))),
                -40.0, 40.0)
    sc = (2.0 ** e).astype(np.float32)                          # [136]
    SCm = np.zeros((128, 2), np.float32)
    SCm[:H0, 0] = sc[:H0]
    SCm[:H1, 1] = sc[H0:136]

    in_maps = []
    for cid in range(NCORES):
        r0 = cid * SH
        Sh = np.ascontiguousarray(
            st8[r0:r0 + SH].reshape(RT, 128, STK).transpose(1, 0, 2)
            .reshape(128, RT * STK))
        in_maps.append({
            "X": _swizzle_shard(L8[r0:r0 + SH, :]),
            "S": Sh,
            "SC": SCm,
        })
    return in_maps, sc


def run_device(L, stack, sy, trace=False):
    """Returns (U [136, N] f32 unscaled, res) computed on 8 NeuronCores."""
    from concourse.bass_utils import run_bass_kernel_spmd
    nc = _get_bass()
    in_maps, sc = _prep_core_inputs(L, stack, sy)
    res = run_bass_kernel_spmd(nc, in_maps, core_ids=list(range(NCORES)),
                               trace=trace)
    U = np.zeros((136, N), np.float64)
    for out in res.results:
        Um = out["U"].astype(np.float64)       # [72, 2N]
        U[:H0] += Um[:, :N]
        U[H0:136] += Um[:H1, N:]
    U /= np.float64(SCALE)
    U /= (sy * sc)[:, None].astype(np.float64)
    return U.astype(np.float32), res


def kernel(**inputs):
    inputs = {k: np.asarray(v) for k, v in inputs.items()}
    L = inputs["L"]
    pr = _host_pre(**inputs)
    stack, sy = _make_stack(pr)
    U, _ = run_device(L, stack, sy)
    return _host_post_q(pr, U, np.diagonal(L).copy())


if __name__ == "__main__":
    import reference
    inputs = reference.setup_inputs()
    y = kernel(**{k: np.asarray(v) for k, v in inputs.items()})
    print("y[0,:5] =", y[0, :5])



# revision 5
# speedup vs baseline: 1.1784x; 1.1784x over previous
"""DNC scatter_memory kernel for 8 Trainium2 NeuronCores.

Strategy: the only heavy tensor is L [8192,8192] (256MB f32). The
returned y depends on L only through a handful of row-contractions of
L, so the device computes a single fused product

  U = S^T @ L   with  S = [s*r, r, M*s]   [8192, 72]

row-sharded over 8 cores (U partials summed on host). Host assembles:

  B1 = L^T(s*r) = U[0:4].T,  B2 = L^T r = U[4:8].T,  V = (M*s)^T L
  Q_fwd = V r - (V(w*r) + Delta) + rank-1/diag fixups
  Q_bwd = M^T B1 - (M*w)^T B2 + rank-1/diag fixups

where Delta = (M*w)^T L (w*r) is O(|w*r|^2)-tiny; the host recovers it
near-exactly from the top-K rows of w (it owns L), so the device needs
only ONE 72-column stationary pass over L instead of the 136-column /
two-pass variant — PE streaming time halves and drops below the HBM
roofline for reading L once.

L is read exactly ONCE per element, as fp8_e4m3 (scaled 2^13; ~1e-4
end-to-end error vs the 2e-2 budget, since every output is an average
of 8192 independently-rounded terms). S is fp8 with per-column-group
power-of-2 scales chosen at runtime; U returns as fp8_e4m3, scaled per
row on-device (tensor_scalar_mul) with host-computed true-bound
power-of-2 scales so the cast can never overflow.

Per core: the 1024-row shard of L*2^13 (8MB fp8) is pre-swizzled on the
host into 16 contiguous 512KB "slabs" [128 part, 8 row-tiles x 512
cols]. One slab = one PSUM-bank accumulation group of 4 fp8-DoubleRow
matmuls (256-row contraction each, operand layout [p, 2, free]). Slabs
rotate through the 8 PSUM banks; ACT copies even banks, DVE odd banks,
both downcasting to fp8 into the obx staging buffer; outputs drain in
overlapping chunks on the sync HWDGE queue behind all input dma_starts
(SEQ-blocking waits can't stall queued input transfers). The last two
slabs ship as quarter-DMAs so only the final matmul trails the last
transfer.

Host side (LSTM controller, interface partition, usage/allocation sort,
content softmaxes, M update, Q assembly, final projection) is tiny.
"""
import numpy as np
import ml_dtypes

N, Wd, R, OUT, IN = 8192, 64, 4, 256, 256
NCORES = 8
SH = N // NCORES          # 1024 rows per core
RT = SH // 128            # 8 row tiles per shard
NSLAB = N // 512          # 16 slabs: one per 512-col chunk
NB = 16                   # slab ring depth (all slabs SBUF-resident)
STK = 80                  # stack row stride (72 cols + pad; %16 for DR)
H0 = 72                   # stationary width: [s*r, r, M*s]
SCALE = 8192.0            # L quantization scale (2^13)
KTOP = 128                # host-side Delta correction rows

F8 = ml_dtypes.float8_e4m3

_cached = {}


def _sig(x):
    return 1.0 / (1.0 + np.exp(-x))


def _softplus(x):
    return np.log1p(np.exp(-np.abs(x))) + np.maximum(x, 0.0)


def _softmax0(x):
    m = x.max(axis=0, keepdims=True)
    e = np.exp(x - m)
    return e / e.sum(axis=0, keepdims=True)


def _l2norm(a, axis):
    return a / np.sqrt(np.maximum((a * a).sum(axis=axis, keepdims=True), 1e-12))


def _host_pre(x, dense_k, dense_b, lstm_k, lstm_r, lstm_b, W_output, W_interface,
              W_read_out, h, c, M, usage, L, W_prec, W_read, W_write, read_v):
    f32 = np.float32
    xd = x.reshape(1, -1) @ dense_k + dense_b
    seq = np.concatenate([xd, read_v], axis=0)
    for t in range(seq.shape[0]):
        z = seq[t][None, :] @ lstm_k + h @ lstm_r + lstm_b
        zi, zf, zg, zo = np.split(z, 4, axis=1)
        c = _sig(zf) * c + _sig(zi) * np.tanh(zg)
        h = _sig(zo) * np.tanh(c)
    out_v = h @ W_output
    iface = (h @ W_interface)[0]

    sizes = [R * Wd, R, Wd, 1, Wd, Wd, R, 1, 1, 3 * R]
    offs = np.cumsum([0] + sizes)
    p = [iface[offs[i]:offs[i + 1]] for i in range(10)]
    k_read = p[0].reshape(R, Wd)
    b_read = 1.0 + _softplus(p[1])[None, :]
    k_write = p[2][None, :]
    b_write = 1.0 + _softplus(p[3])[None, :]
    erase = _sig(p[4])[None, :]
    write_v = p[5][None, :]
    free_gates = _sig(p[6])[None, :]
    alloc_gate = _sig(p[7][0])
    write_gate = _sig(p[8][0])
    read_modes = _softmax0(p[9].reshape(3, R))

    retention = np.prod(1.0 - free_gates * W_read, axis=1, keepdims=True).astype(f32)
    usage = ((usage + W_write - usage * W_write) * retention).astype(f32)
    u_flat = usage[:, 0]
    order = np.argsort(u_flat, kind="stable")
    sorted_u = u_flat[order]
    cp = np.cumprod(sorted_u).astype(f32)
    cp_excl = np.concatenate([np.ones((1,), f32), cp[:-1]])
    unorder = (1.0 - sorted_u) * cp_excl
    W_alloc = np.zeros((N,), f32)
    W_alloc[order] = unorder
    W_alloc = W_alloc[:, None]

    nM = _l2norm(M, 1)
    W_lookup_w = _softmax0(nM @ _l2norm(k_write, 1).T * b_write)
    W_write_new = (write_gate * (alloc_gate * W_alloc
                                 + (1.0 - alloc_gate) * W_lookup_w)).astype(f32)
    M_new = (M * (1.0 - W_write_new @ erase) + W_write_new @ write_v).astype(f32)

    w = W_write_new[:, 0]
    s = (1.0 - w).astype(f32)
    r = W_read.astype(f32)
    return dict(out_v=out_v.astype(f32), k_read=k_read, b_read=b_read,
                read_modes=read_modes, M_new=M_new, w=w, s=s, r=r,
                W_read_out=W_read_out, W_prec=W_prec)


def _make_stack(pr):
    """[N, 72] f32 stack S = [s*r, r, M*s] and per-column scales."""
    w, s, r, M = pr["w"], pr["s"], pr["r"], pr["M_new"]
    stack = np.concatenate(
        [s[:, None] * r, r, M * s[:, None]], axis=1).astype(np.float32)
    sy = np.empty(H0, np.float32)
    for cols in (slice(0, 8), slice(8, H0)):
        amax = float(np.abs(stack[:, cols]).max())
        e = np.clip(np.floor(np.log2(160.0 / max(amax, 1e-30))), -40.0, 40.0)
        sy[cols] = 2.0 ** e
    return stack, sy


def _host_post_q(pr, U, diagL, L):
    """Assemble y from U = S^T @ L [72, N] (f32, unscaled)."""
    w, s, r = pr["w"], pr["s"], pr["r"]
    p = pr["W_prec"][:, 0]
    M = pr["M_new"]
    Ld_new = ((1.0 - 2.0 * w) * diagL + w * p).astype(np.float32)

    B1 = U[0:4].T                       # L^T (s*r)   [N,4]
    B2 = U[4:8].T                       # L^T r       [N,4]
    V = U[8:H0]                         # (M*s)^T L   [64,N]
    wr = (w[:, None] * r).astype(np.float32)
    G1 = V @ r                          # (M*s)^T L r        [64,4]
    # G2 = M^T L (w*r) = V(w*r) + (M*w)^T L (w*r); the second term is
    # O(|w*r|^2)-tiny and dominated by the largest-w rows — recover it
    # exactly for the top-K rows from host-resident L.
    G2 = V @ wr
    idx = np.argpartition(w, N - KTOP)[N - KTOP:]
    q = L[idx, :].astype(np.float32) @ wr
    G2 = G2 + (M[idx] * w[idx, None]).T @ q

    pr_r = p @ r
    wr_r = w @ r
    Mld_r = (M * Ld_new[:, None]).T @ r
    Q_fwd = G1 - G2 + np.outer(M.T @ w, pr_r) - Mld_r
    Q_bwd = (M.T @ B1) - ((M * w[:, None]).T @ B2) \
        + np.outer(M.T @ p, wr_r) - Mld_r

    W_lookup_r = _softmax0(_l2norm(M, 1) @ _l2norm(pr["k_read"], 1).T
                           * pr["b_read"])
    Q_look = M.T @ W_lookup_r
    modes = pr["read_modes"]
    Q = modes[0][None, :] * Q_bwd + modes[1][None, :] * Q_look \
        + modes[2][None, :] * Q_fwd                     # [64, 4]

    read_v_new = Q.T                                    # [4, 64]
    y = pr["out_v"] + read_v_new.reshape(1, R * Wd) @ pr["W_read_out"]
    return y.astype(np.float32)


def _build_bass():
    """Raw-Bass SPMD kernel: U = S^T @ L, single row-layout fp8 pass.

    Inputs per core (host pre-swizzled):
      X  [NSLAB*128, 4096] fp8: slab s at rows [128s : 128s+128], SBUF
         partition p byte 512t+c = shard element [row 128t+p, col
         512s+c] of the L row shard.
      S  [128, RT*STK] fp8: partition p, element STK*t+j = stack column
         j of shard row 128t+p (j<72: [s*r, r, M*s]; 72:80 zero pad so
         the DoubleRow middle-dim stride is 16B-aligned).
      SC [128, 1] f32: per-U-row output scales (rows 0:72).
    One slab = one PSUM accumulation group of 4 fp8 DoubleRow matmuls
    (start->stop full bank lifecycle) rotating over the 8 banks. ACT
    copies even banks, DVE odd banks (they may touch PSUM concurrently
    on different banks), each scaling by SC and casting to fp8 into
    obx. Outputs drain in chunks on the sync HWDGE queue after all
    input dma_starts, overlapping the tail of the input stream; the
    last two slabs ship as quarter-DMAs (one per DoubleRow matmul's
    stripe) so only the final matmul trails the last transfer.
    """
    import concourse.bass as bass
    import concourse.mybir as mybir
    from contextlib import ExitStack

    f32 = mybir.dt.float32
    f8 = mybir.dt.float8e4
    nc = bass.Bass()
    X = nc.dram_tensor("X", [NSLAB * 128, 4096], f8, kind="ExternalInput")
    S = nc.dram_tensor("S", [128, RT * STK], f8, kind="ExternalInput")
    SC = nc.dram_tensor("SC", [128, 1], f32, kind="ExternalInput")
    U = nc.dram_tensor("U", [H0, N], f8, kind="ExternalOutput")

    with ExitStack() as es:
        xb = es.enter_context(nc.sbuf_tensor("xb", [128, NB * 4096], f8))
        sb = es.enter_context(nc.sbuf_tensor("sb", [128, RT * STK], f8))
        scb = es.enter_context(nc.sbuf_tensor("scb", [128, 1], f32))
        obx = es.enter_context(nc.sbuf_tensor("obx", [H0, NSLAB * 512], f8))
        ps = [es.enter_context(nc.psum_tensor(f"ps{b}", [H0, 512], f32))
              for b in range(8)]
        sem_s = es.enter_context(nc.semaphore(name="sem_s"))
        slot = [es.enter_context(nc.semaphore(name=f"slot{i}"))
                for i in range(NSLAB - 2)]
        sem_q = [[es.enter_context(nc.semaphore(name=f"sem_q{j}_{i}"))
                  for i in range(RT // 2)] for j in range(2)]
        pe_sem = es.enter_context(nc.semaphore(name="pe_sem"))
        act_c = es.enter_context(nc.semaphore(name="act_c"))
        dve_c = es.enter_context(nc.semaphore(name="dve_c"))
        out_sem = es.enter_context(nc.semaphore(name="out_sem"))
        blk = es.enter_context(nc.Block())

        @blk.scalar
        def _(scalar):
            # stack + scale loads on the ACT HWDGE queue (parallel with
            # the X stream on the sync queue), then even-bank copies
            scalar.dma_start(out=sb[:, :], in_=S[:, :]).then_inc(sem_s, 16)
            scalar.dma_start(out=scb[:, :], in_=SC[:, :]).then_inc(sem_s, 16)
            scalar.wait_ge(sem_s, 32)
            for g in range(0, NSLAB, 2):
                scalar.wait_ge(pe_sem, g + 1)
                scalar.mul(obx[:, 512 * g:512 * (g + 1)],
                           ps[g % 8][:, :],
                           scb[:H0, 0:1]).then_inc(act_c, 1)

        @blk.sync
        def _(sync):
            for s in range(NSLAB):
                k = s % NB
                if s >= NSLAB - 2:
                    # quarter-DMAs (one per DoubleRow matmul's stripe) so
                    # only the final matmul pair trails the last transfer
                    for qq in range(RT // 2):
                        sync.dma_start(
                            out=xb[:, 4096 * k + 1024 * qq:
                                   4096 * k + 1024 * (qq + 1)],
                            in_=X[128 * s:128 * (s + 1),
                                  1024 * qq:1024 * (qq + 1)],
                        ).then_inc(sem_q[s - (NSLAB - 2)][qq], 16)
                    continue
                sync.dma_start(
                    out=xb[:, 4096 * k:4096 * (k + 1)],
                    in_=X[128 * s:128 * (s + 1), :],
                ).then_inc(slot[s], 16)
            # outputs behind all queued inputs; chunks get finer toward
            # the end so the last transfers are small and fire early
            for s0, s1 in ((0, 8), (8, 12), (12, 14), (14, 15), (15, 16)):
                sync.wait_ge(act_c, (s1 + 1) // 2)
                sync.wait_ge(dve_c, s1 // 2)
                sync.dma_start(
                    out=U[:, 512 * s0:512 * s1],
                    in_=obx[:, 512 * s0:512 * s1],
                ).then_inc(out_sem, 16)

        @blk.tensor
        def _(tensor):
            tensor.wait_ge(sem_s, 16)   # stationary resident (S only)
            sbr = sb.rearrange("p (t j) -> p t j", j=STK)
            xbr = xb.rearrange("p (q c) -> p q c", c=512)

            def mk_mm(s, k, u):
                mm = tensor.matmul(
                    ps[s % 8][:, :],
                    sbr[:, 2 * u:2 * u + 2, 0:H0],
                    xbr[:, 8 * k + 2 * u:8 * k + 2 * u + 2, :],
                    start=(u == 0), stop=(u == RT // 2 - 1),
                    perf_mode=mybir.MatmulPerfMode.DoubleRow,
                )
                if u == RT // 2 - 1:
                    mm.then_inc(pe_sem, 1)

            for s in range(NSLAB):
                k = s % NB
                if s >= 8:   # bank reuse: copier drained group s-8
                    if (s - 8) % 2 == 0:
                        tensor.wait_ge(act_c, (s - 8) // 2 + 1)
                    else:
                        tensor.wait_ge(dve_c, (s - 8) // 2 + 1)
                if s < NSLAB - 2:
                    tensor.wait_ge(slot[s], 16)
                    for u in range(RT // 2):
                        mk_mm(s, k, u)
                else:
                    for u in range(RT // 2):
                        tensor.wait_ge(sem_q[s - (NSLAB - 2)][u], 16)
                        mk_mm(s, k, u)

        @blk.vector
        def _(vector):
            vector.wait_ge(sem_s, 32)
            for g in range(1, NSLAB, 2):    # odd banks on DVE
                vector.wait_ge(pe_sem, g + 1)
                vector.tensor_scalar_mul(
                    obx[:, 512 * g:512 * (g + 1)],
                    ps[g % 8][:, :],
                    scb[:H0, 0:1],
                ).then_inc(dve_c, 1)

    return nc


def _get_bass():
    if "nc" not in _cached:
        _cached["nc"] = _build_bass()
    return _cached["nc"]


def _swizzle_shard(S8):
    """[1024, 8192] fp8 shard -> [16*128, 4096] slab-major layout."""
    # want A[cc, p, t, c] = S8[128t + p, 512cc + c]
    a = S8.reshape(RT, 128, NSLAB, 512).transpose(2, 1, 0, 3)
    return np.ascontiguousarray(a).reshape(NSLAB * 128, RT * 512)


def _prep_core_inputs(L, stack, sy):
    """Quantize L (scaled) + stack to fp8, build per-core input maps.

    Also computes per-U-row power-of-2 output scales from the TRUE bound
    |U[j,:]| <= sum_i |S_q[i,j]| * max|L_q| (x1.14 quantization margin),
    so the on-device fp8 cast of scaled PSUM rows can never overflow.
    """
    L8 = (L * np.float32(SCALE)).astype(F8)
    scaled_f = stack * sy[None, :]
    scaled = scaled_f.astype(F8)
    st8 = np.zeros((N, STK), F8)
    st8[:, :H0] = scaled

    lmax = float(np.abs(L).max()) * SCALE
    bound = np.abs(scaled_f).sum(axis=0) * lmax * 1.14          # [72]
    e = np.clip(np.floor(np.log2(240.0 / np.maximum(bound, 1e-30))),
                -40.0, 40.0)
    sc = (2.0 ** e).astype(np.float32)                          # [72]
    SCm = np.zeros((128, 1), np.float32)
    SCm[:H0, 0] = sc

    in_maps = []
    for cid in range(NCORES):
        r0 = cid * SH
        Sh = np.ascontiguousarray(
            st8[r0:r0 + SH].reshape(RT, 128, STK).transpose(1, 0, 2)
            .reshape(128, RT * STK))
        in_maps.append({
            "X": _swizzle_shard(L8[r0:r0 + SH, :]),
            "S": Sh,
            "SC": SCm,
        })
    return in_maps, sc


def run_device(L, stack, sy, trace=False):
    """Returns (U [72, N] f32 unscaled, res) computed on 8 NeuronCores."""
    from concourse.bass_utils import run_bass_kernel_spmd
    nc = _get_bass()
    in_maps, sc = _prep_core_inputs(L, stack, sy)
    res = run_bass_kernel_spmd(nc, in_maps, core_ids=list(range(NCORES)),
                               trace=trace)
    U = np.zeros((H0, N), np.float64)
    for out in res.results:
        U += out["U"].astype(np.float64)
    U /= np.float64(SCALE)
    U /= (sy * sc)[:, None].astype(np.float64)
    return U.astype(np.float32), res


def kernel(**inputs):
    inputs = {k: np.asarray(v) for k, v in inputs.items()}
    L = inputs["L"]
    pr = _host_pre(**inputs)
    stack, sy = _make_stack(pr)
    U, _ = run_device(L, stack, sy)
    return _host_post_q(pr, U, np.diagonal(L).copy(), L)


if __name__ == "__main__":
    import reference
    inputs = reference.setup_inputs()
    y = kernel(**{k: np.asarray(v) for k, v in inputs.items()})
    print("y[0,:5] =", y[0, :5])


# revision 8
# speedup vs baseline: 1.1802x; 1.0016x over previous
"""DNC scatter_memory kernel for 8 Trainium2 NeuronCores.

Strategy: the only heavy tensor is L [8192,8192] (256MB f32). The
returned y depends on L only through a handful of row-contractions of
L, so the device computes a single fused product

  U = S^T @ L   with  S = [s*r, r, M*s]   [8192, 72]

row-sharded over 8 cores (U partials summed on host). Host assembles:

  B1 = L^T(s*r) = U[0:4].T,  B2 = L^T r = U[4:8].T,  V = (M*s)^T L
  Q_fwd = V r - (V(w*r) + Delta) + rank-1/diag fixups
  Q_bwd = M^T B1 - (M*w)^T B2 + rank-1/diag fixups

where Delta = (M*w)^T L (w*r) is O(|w*r|^2)-tiny; the host recovers it
near-exactly from the top-K rows of w (it owns L), so the device needs
only ONE 72-column stationary pass over L instead of the 136-column /
two-pass variant — PE streaming time halves and drops below the HBM
roofline for reading L once.

L is read exactly ONCE per element, as fp8_e4m3 (scaled 2^13; ~1e-4
end-to-end error vs the 2e-2 budget, since every output is an average
of 8192 independently-rounded terms). S is fp8 with per-column-group
power-of-2 scales chosen at runtime; U returns as fp8_e4m3, scaled per
row on-device (tensor_scalar_mul) with host-computed true-bound
power-of-2 scales so the cast can never overflow.

Per core: the 1024-row shard of L*2^13 (8MB fp8) is pre-swizzled on the
host into 16 contiguous 512KB "slabs" [128 part, 8 row-tiles x 512
cols]. One slab = one PSUM-bank accumulation group of 4 fp8-DoubleRow
matmuls (256-row contraction each, operand layout [p, 2, free]). Slabs
rotate through the 8 PSUM banks; ACT copies even banks, DVE odd banks,
both downcasting to fp8 into the obx staging buffer; outputs drain in
overlapping chunks on the sync HWDGE queue behind all input dma_starts
(SEQ-blocking waits can't stall queued input transfers). The last two
slabs ship as quarter-DMAs so only the final matmul trails the last
transfer.

Host side (LSTM controller, interface partition, usage/allocation sort,
content softmaxes, M update, Q assembly, final projection) is tiny.
"""
import numpy as np
import ml_dtypes

N, Wd, R, OUT, IN = 8192, 64, 4, 256, 256
NCORES = 8
SH = N // NCORES          # 1024 rows per core
RT = SH // 128            # 8 row tiles per shard
NSLAB = N // 512          # 16 slabs: one per 512-col chunk
NB = 16                   # slab ring depth (all slabs SBUF-resident)
STK = 80                  # stack row stride (72 cols + pad; %16 for DR)
H0 = 72                   # stationary width: [s*r, r, M*s]
SCALE = 8192.0            # L quantization scale (2^13)
KTOP = 128                # host-side Delta correction rows

F8 = ml_dtypes.float8_e4m3

_cached = {}


def _sig(x):
    return 1.0 / (1.0 + np.exp(-x))


def _softplus(x):
    return np.log1p(np.exp(-np.abs(x))) + np.maximum(x, 0.0)


def _softmax0(x):
    m = x.max(axis=0, keepdims=True)
    e = np.exp(x - m)
    return e / e.sum(axis=0, keepdims=True)


def _l2norm(a, axis):
    return a / np.sqrt(np.maximum((a * a).sum(axis=axis, keepdims=True), 1e-12))


def _host_pre(x, dense_k, dense_b, lstm_k, lstm_r, lstm_b, W_output, W_interface,
              W_read_out, h, c, M, usage, L, W_prec, W_read, W_write, read_v):
    f32 = np.float32
    xd = x.reshape(1, -1) @ dense_k + dense_b
    seq = np.concatenate([xd, read_v], axis=0)
    for t in range(seq.shape[0]):
        z = seq[t][None, :] @ lstm_k + h @ lstm_r + lstm_b
        zi, zf, zg, zo = np.split(z, 4, axis=1)
        c = _sig(zf) * c + _sig(zi) * np.tanh(zg)
        h = _sig(zo) * np.tanh(c)
    out_v = h @ W_output
    iface = (h @ W_interface)[0]

    sizes = [R * Wd, R, Wd, 1, Wd, Wd, R, 1, 1, 3 * R]
    offs = np.cumsum([0] + sizes)
    p = [iface[offs[i]:offs[i + 1]] for i in range(10)]
    k_read = p[0].reshape(R, Wd)
    b_read = 1.0 + _softplus(p[1])[None, :]
    k_write = p[2][None, :]
    b_write = 1.0 + _softplus(p[3])[None, :]
    erase = _sig(p[4])[None, :]
    write_v = p[5][None, :]
    free_gates = _sig(p[6])[None, :]
    alloc_gate = _sig(p[7][0])
    write_gate = _sig(p[8][0])
    read_modes = _softmax0(p[9].reshape(3, R))

    retention = np.prod(1.0 - free_gates * W_read, axis=1, keepdims=True).astype(f32)
    usage = ((usage + W_write - usage * W_write) * retention).astype(f32)
    u_flat = usage[:, 0]
    order = np.argsort(u_flat, kind="stable")
    sorted_u = u_flat[order]
    cp = np.cumprod(sorted_u).astype(f32)
    cp_excl = np.concatenate([np.ones((1,), f32), cp[:-1]])
    unorder = (1.0 - sorted_u) * cp_excl
    W_alloc = np.zeros((N,), f32)
    W_alloc[order] = unorder
    W_alloc = W_alloc[:, None]

    nM = _l2norm(M, 1)
    W_lookup_w = _softmax0(nM @ _l2norm(k_write, 1).T * b_write)
    W_write_new = (write_gate * (alloc_gate * W_alloc
                                 + (1.0 - alloc_gate) * W_lookup_w)).astype(f32)
    M_new = (M * (1.0 - W_write_new @ erase) + W_write_new @ write_v).astype(f32)

    w = W_write_new[:, 0]
    s = (1.0 - w).astype(f32)
    r = W_read.astype(f32)
    return dict(out_v=out_v.astype(f32), k_read=k_read, b_read=b_read,
                read_modes=read_modes, M_new=M_new, w=w, s=s, r=r,
                W_read_out=W_read_out, W_prec=W_prec)


def _make_stack(pr):
    """[N, 72] f32 stack S = [s*r, r, M*s] and per-column scales."""
    w, s, r, M = pr["w"], pr["s"], pr["r"], pr["M_new"]
    stack = np.concatenate(
        [s[:, None] * r, r, M * s[:, None]], axis=1).astype(np.float32)
    sy = np.empty(H0, np.float32)
    for cols in (slice(0, 8), slice(8, H0)):
        amax = float(np.abs(stack[:, cols]).max())
        e = np.clip(np.floor(np.log2(160.0 / max(amax, 1e-30))), -40.0, 40.0)
        sy[cols] = 2.0 ** e
    return stack, sy


def _host_post_q(pr, U, diagL, L):
    """Assemble y from U = S^T @ L [72, N] (f32, unscaled)."""
    w, s, r = pr["w"], pr["s"], pr["r"]
    p = pr["W_prec"][:, 0]
    M = pr["M_new"]
    Ld_new = ((1.0 - 2.0 * w) * diagL + w * p).astype(np.float32)

    B1 = U[0:4].T                       # L^T (s*r)   [N,4]
    B2 = U[4:8].T                       # L^T r       [N,4]
    V = U[8:H0]                         # (M*s)^T L   [64,N]
    wr = (w[:, None] * r).astype(np.float32)
    G1 = V @ r                          # (M*s)^T L r        [64,4]
    # G2 = M^T L (w*r) = V(w*r) + (M*w)^T L (w*r); the second term is
    # O(|w*r|^2)-tiny and dominated by the largest-w rows — recover it
    # exactly for the top-K rows from host-resident L.
    G2 = V @ wr
    idx = np.argpartition(w, N - KTOP)[N - KTOP:]
    q = L[idx, :].astype(np.float32) @ wr
    G2 = G2 + (M[idx] * w[idx, None]).T @ q

    pr_r = p @ r
    wr_r = w @ r
    Mld_r = (M * Ld_new[:, None]).T @ r
    Q_fwd = G1 - G2 + np.outer(M.T @ w, pr_r) - Mld_r
    Q_bwd = (M.T @ B1) - ((M * w[:, None]).T @ B2) \
        + np.outer(M.T @ p, wr_r) - Mld_r

    W_lookup_r = _softmax0(_l2norm(M, 1) @ _l2norm(pr["k_read"], 1).T
                           * pr["b_read"])
    Q_look = M.T @ W_lookup_r
    modes = pr["read_modes"]
    Q = modes[0][None, :] * Q_bwd + modes[1][None, :] * Q_look \
        + modes[2][None, :] * Q_fwd                     # [64, 4]

    read_v_new = Q.T                                    # [4, 64]
    y = pr["out_v"] + read_v_new.reshape(1, R * Wd) @ pr["W_read_out"]
    return y.astype(np.float32)


def _build_bass():
    """Raw-Bass SPMD kernel: U = S^T @ L, single row-layout fp8 pass.

    Inputs per core (host pre-swizzled):
      X  [NSLAB*128, 4096] fp8: slab s at rows [128s : 128s+128], SBUF
         partition p byte 512t+c = shard element [row 128t+p, col
         512s+c] of the L row shard.
      S  [128, RT*STK] fp8: partition p, element STK*t+j = stack column
         j of shard row 128t+p (j<72: [s*r, r, M*s]; 72:80 zero pad so
         the DoubleRow middle-dim stride is 16B-aligned).
      SC [128, 1] f32: per-U-row output scales (rows 0:72).
    One slab = one PSUM accumulation group of 4 fp8 DoubleRow matmuls
    (start->stop full bank lifecycle) rotating over the 8 banks. ACT
    copies even banks, DVE odd banks (they may touch PSUM concurrently
    on different banks), each scaling by SC and casting to fp8 into
    obx. Outputs drain in chunks on the sync HWDGE queue after all
    input dma_starts, overlapping the tail of the input stream; the
    last two slabs ship as quarter-DMAs (one per DoubleRow matmul's
    stripe) so only the final matmul trails the last transfer.
    """
    import concourse.bass as bass
    import concourse.mybir as mybir
    from contextlib import ExitStack

    f32 = mybir.dt.float32
    f8 = mybir.dt.float8e4
    nc = bass.Bass()
    X = nc.dram_tensor("X", [NSLAB * 128, 4096], f8, kind="ExternalInput")
    S = nc.dram_tensor("S", [128, RT * STK], f8, kind="ExternalInput")
    SC = nc.dram_tensor("SC", [128, 1], f32, kind="ExternalInput")
    U = nc.dram_tensor("U", [H0, N], f8, kind="ExternalOutput")

    with ExitStack() as es:
        xb = es.enter_context(nc.sbuf_tensor("xb", [128, NB * 4096], f8))
        sb = es.enter_context(nc.sbuf_tensor("sb", [128, RT * STK], f8))
        scb = es.enter_context(nc.sbuf_tensor("scb", [128, 1], f32))
        obx = es.enter_context(nc.sbuf_tensor("obx", [H0, NSLAB * 512], f8))
        ps = [es.enter_context(nc.psum_tensor(f"ps{b}", [H0, 512], f32))
              for b in range(8)]
        sem_s = es.enter_context(nc.semaphore(name="sem_s"))
        sem_sc = es.enter_context(nc.semaphore(name="sem_sc"))
        slot = [es.enter_context(nc.semaphore(name=f"slot{i}"))
                for i in range(NSLAB - 2)]
        sem_q = [[es.enter_context(nc.semaphore(name=f"sem_q{j}_{i}"))
                  for i in range(RT // 2)] for j in range(2)]
        pe_sem = es.enter_context(nc.semaphore(name="pe_sem"))
        act_c = es.enter_context(nc.semaphore(name="act_c"))
        dve_c = es.enter_context(nc.semaphore(name="dve_c"))
        out_sem = es.enter_context(nc.semaphore(name="out_sem"))
        blk = es.enter_context(nc.Block())

        @blk.scalar
        def _(scalar):
            # stack + scale loads on the ACT HWDGE queue (parallel with
            # the X stream on the sync queue), then even-bank copies
            scalar.dma_start(out=sb[:, :], in_=S[:, :]).then_inc(sem_s, 16)
            scalar.dma_start(out=scb[:, :], in_=SC[:, :]).then_inc(sem_sc, 16)
            scalar.wait_ge(sem_sc, 16)
            for g in range(0, NSLAB, 2):
                scalar.wait_ge(pe_sem, g + 1)
                scalar.mul(obx[:, 512 * g:512 * (g + 1)],
                           ps[g % 8][:, :],
                           scb[:H0, 0:1]).then_inc(act_c, 1)

        @blk.sync
        def _(sync):
            for s in range(NSLAB):
                k = s % NB
                if s >= NSLAB - 2:
                    # quarter-DMAs (one per DoubleRow matmul's stripe) so
                    # only the final matmul pair trails the last transfer
                    for qq in range(RT // 2):
                        sync.dma_start(
                            out=xb[:, 4096 * k + 1024 * qq:
                                   4096 * k + 1024 * (qq + 1)],
                            in_=X[128 * s:128 * (s + 1),
                                  1024 * qq:1024 * (qq + 1)],
                        ).then_inc(sem_q[s - (NSLAB - 2)][qq], 16)
                    continue
                sync.dma_start(
                    out=xb[:, 4096 * k:4096 * (k + 1)],
                    in_=X[128 * s:128 * (s + 1), :],
                ).then_inc(slot[s], 16)
            # outputs behind all queued inputs; chunks get finer toward
            # the end so the last transfers are small and fire early
            for s0, s1 in ((0, 8), (8, 12), (12, 14), (14, 15), (15, 16)):
                sync.wait_ge(act_c, (s1 + 1) // 2)
                sync.wait_ge(dve_c, s1 // 2)
                sync.dma_start(
                    out=U[:, 512 * s0:512 * s1],
                    in_=obx[:, 512 * s0:512 * s1],
                ).then_inc(out_sem, 16)

        @blk.tensor
        def _(tensor):
            tensor.wait_ge(sem_s, 16)   # stationary resident (S only)
            sbr = sb.rearrange("p (t j) -> p t j", j=STK)
            xbr = xb.rearrange("p (q c) -> p q c", c=512)

            def mk_mm(s, k, u):
                mm = tensor.matmul(
                    ps[s % 8][:, :],
                    sbr[:, 2 * u:2 * u + 2, 0:H0],
                    xbr[:, 8 * k + 2 * u:8 * k + 2 * u + 2, :],
                    start=(u == 0), stop=(u == RT // 2 - 1),
                    perf_mode=mybir.MatmulPerfMode.DoubleRow,
                )
                if u == RT // 2 - 1:
                    mm.then_inc(pe_sem, 1)

            for s in range(NSLAB):
                k = s % NB
                if s >= 8:   # bank reuse: copier drained group s-8
                    if (s - 8) % 2 == 0:
                        tensor.wait_ge(act_c, (s - 8) // 2 + 1)
                    else:
                        tensor.wait_ge(dve_c, (s - 8) // 2 + 1)
                if s < NSLAB - 2:
                    tensor.wait_ge(slot[s], 16)
                    for u in range(RT // 2):
                        mk_mm(s, k, u)
                else:
                    for u in range(RT // 2):
                        tensor.wait_ge(sem_q[s - (NSLAB - 2)][u], 16)
                        mk_mm(s, k, u)

        @blk.vector
        def _(vector):
            vector.wait_ge(sem_sc, 16)
            for g in range(1, NSLAB, 2):    # odd banks on DVE
                vector.wait_ge(pe_sem, g + 1)
                vector.tensor_scalar_mul(
                    obx[:, 512 * g:512 * (g + 1)],
                    ps[g % 8][:, :],
                    scb[:H0, 0:1],
                ).then_inc(dve_c, 1)

    return nc


def _get_bass():
    if "nc" not in _cached:
        _cached["nc"] = _build_bass()
    return _cached["nc"]


def _swizzle_shard(S8):
    """[1024, 8192] fp8 shard -> [16*128, 4096] slab-major layout."""
    # want A[cc, p, t, c] = S8[128t + p, 512cc + c]
    a = S8.reshape(RT, 128, NSLAB, 512).transpose(2, 1, 0, 3)
    return np.ascontiguousarray(a).reshape(NSLAB * 128, RT * 512)


def _prep_core_inputs(L, stack, sy):
    """Quantize L (scaled) + stack to fp8, build per-core input maps.

    Also computes per-U-row power-of-2 output scales from the TRUE bound
    |U[j,:]| <= sum_i |S_q[i,j]| * max|L_q| (x1.14 quantization margin),
    so the on-device fp8 cast of scaled PSUM rows can never overflow.
    """
    L8 = (L * np.float32(SCALE)).astype(F8)
    scaled_f = stack * sy[None, :]
    scaled = scaled_f.astype(F8)
    st8 = np.zeros((N, STK), F8)
    st8[:, :H0] = scaled

    lmax = float(np.abs(L).max()) * SCALE
    bound = np.abs(scaled_f).sum(axis=0) * lmax * 1.14          # [72]
    e = np.clip(np.floor(np.log2(240.0 / np.maximum(bound, 1e-30))),
                -40.0, 40.0)
    sc = (2.0 ** e).astype(np.float32)                          # [72]
    SCm = np.zeros((128, 1), np.float32)
    SCm[:H0, 0] = sc

    in_maps = []
    for cid in range(NCORES):
        r0 = cid * SH
        Sh = np.ascontiguousarray(
            st8[r0:r0 + SH].reshape(RT, 128, STK).transpose(1, 0, 2)
            .reshape(128, RT * STK))
        in_maps.append({
            "X": _swizzle_shard(L8[r0:r0 + SH, :]),
            "S": Sh,
            "SC": SCm,
        })
    return in_maps, sc


def run_device(L, stack, sy, trace=False):
    """Returns (U [72, N] f32 unscaled, res) computed on 8 NeuronCores."""
    from concourse.bass_utils import run_bass_kernel_spmd
    nc = _get_bass()
    in_maps, sc = _prep_core_inputs(L, stack, sy)
    res = run_bass_kernel_spmd(nc, in_maps, core_ids=list(range(NCORES)),
                               trace=trace)
    U = np.zeros((H0, N), np.float64)
    for out in res.results:
        U += out["U"].astype(np.float64)
    U /= np.float64(SCALE)
    U /= (sy * sc)[:, None].astype(np.float64)
    return U.astype(np.float32), res


def kernel(**inputs):
    inputs = {k: np.asarray(v) for k, v in inputs.items()}
    L = inputs["L"]
    pr = _host_pre(**inputs)
    stack, sy = _make_stack(pr)
    U, _ = run_device(L, stack, sy)
    return _host_post_q(pr, U, np.diagonal(L).copy(), L)


if __name__ == "__main__":
    import reference
    inputs = reference.setup_inputs()
    y = kernel(**{k: np.asarray(v) for k, v in inputs.items()})
    print("y[0,:5] =", y[0, :5])


# revision 13
# speedup vs baseline: 1.2269x; 1.0395x over previous
"""DNC scatter_memory kernel for 8 Trainium2 NeuronCores.

Strategy: the only heavy tensor is L [8192,8192] (256MB f32). The
returned y depends on L only through a handful of row-contractions of
L, so the device computes a single fused product

  U = S^T @ L   with  S = [s*r, r, M*s]   [8192, 72]

row-sharded over 8 cores (U partials summed on host). Host assembles:

  B1 = L^T(s*r) = U[0:4].T,  B2 = L^T r = U[4:8].T,  V = (M*s)^T L
  Q_fwd = V r - (V(w*r) + Delta) + rank-1/diag fixups
  Q_bwd = M^T B1 - (M*w)^T B2 + rank-1/diag fixups

where Delta = (M*w)^T L (w*r) is O(|w*r|^2)-tiny; the host recovers it
near-exactly from the top-K rows of w (it owns L), so the device needs
only ONE 72-column stationary pass over L instead of the 136-column /
two-pass variant — PE streaming time halves and drops below the HBM
roofline for reading L once.

L is read exactly ONCE per element, as fp8_e4m3 (scaled 2^13; ~1e-4
end-to-end error vs the 2e-2 budget, since every output is an average
of 8192 independently-rounded terms). S is fp8 with per-column-group
power-of-2 scales chosen at runtime; U returns as fp8_e4m3, scaled per
row on-device (tensor_scalar_mul) with host-computed true-bound
power-of-2 scales so the cast can never overflow.

Per core: the 1024-row shard of L*2^13 (8MB fp8) is pre-swizzled on the
host into 16 contiguous 512KB "slabs" [128 part, 8 row-tiles x 512
cols]. One slab = one PSUM-bank accumulation group of 4 fp8-DoubleRow
matmuls (256-row contraction each, operand layout [p, 2, free]). Slabs
rotate through the 8 PSUM banks; ACT copies even banks, DVE odd banks,
both downcasting to fp8 into the obx staging buffer; outputs drain in
overlapping chunks on the sync HWDGE queue behind all input dma_starts
(SEQ-blocking waits can't stall queued input transfers). The last two
slabs ship as quarter-DMAs so only the final matmul trails the last
transfer.

Host side (LSTM controller, interface partition, usage/allocation sort,
content softmaxes, M update, Q assembly, final projection) is tiny.
"""
import numpy as np
import ml_dtypes

N, Wd, R, OUT, IN = 8192, 64, 4, 256, 256
NCORES = 8
SH = N // NCORES          # 1024 rows per core
RT = SH // 128            # 8 row tiles per shard
NSLAB = N // 512          # 16 slabs: one per 512-col chunk
NB = 16                   # slab ring depth (all slabs SBUF-resident)
STK = 80                  # stack row stride (72 cols + pad; %16 for DR)
H0 = 72                   # stationary width: [s*r, r, M*s]
SCALE = 8192.0            # L quantization scale (2^13)
KTOP = 128                # host-side Delta correction rows

F8 = ml_dtypes.float8_e4m3

_cached = {}


def _sig(x):
    return 1.0 / (1.0 + np.exp(-x))


def _softplus(x):
    return np.log1p(np.exp(-np.abs(x))) + np.maximum(x, 0.0)


def _softmax0(x):
    m = x.max(axis=0, keepdims=True)
    e = np.exp(x - m)
    return e / e.sum(axis=0, keepdims=True)


def _l2norm(a, axis):
    return a / np.sqrt(np.maximum((a * a).sum(axis=axis, keepdims=True), 1e-12))


def _host_pre(x, dense_k, dense_b, lstm_k, lstm_r, lstm_b, W_output, W_interface,
              W_read_out, h, c, M, usage, L, W_prec, W_read, W_write, read_v):
    f32 = np.float32
    xd = x.reshape(1, -1) @ dense_k + dense_b
    seq = np.concatenate([xd, read_v], axis=0)
    for t in range(seq.shape[0]):
        z = seq[t][None, :] @ lstm_k + h @ lstm_r + lstm_b
        zi, zf, zg, zo = np.split(z, 4, axis=1)
        c = _sig(zf) * c + _sig(zi) * np.tanh(zg)
        h = _sig(zo) * np.tanh(c)
    out_v = h @ W_output
    iface = (h @ W_interface)[0]

    sizes = [R * Wd, R, Wd, 1, Wd, Wd, R, 1, 1, 3 * R]
    offs = np.cumsum([0] + sizes)
    p = [iface[offs[i]:offs[i + 1]] for i in range(10)]
    k_read = p[0].reshape(R, Wd)
    b_read = 1.0 + _softplus(p[1])[None, :]
    k_write = p[2][None, :]
    b_write = 1.0 + _softplus(p[3])[None, :]
    erase = _sig(p[4])[None, :]
    write_v = p[5][None, :]
    free_gates = _sig(p[6])[None, :]
    alloc_gate = _sig(p[7][0])
    write_gate = _sig(p[8][0])
    read_modes = _softmax0(p[9].reshape(3, R))

    retention = np.prod(1.0 - free_gates * W_read, axis=1, keepdims=True).astype(f32)
    usage = ((usage + W_write - usage * W_write) * retention).astype(f32)
    u_flat = usage[:, 0]
    order = np.argsort(u_flat, kind="stable")
    sorted_u = u_flat[order]
    cp = np.cumprod(sorted_u).astype(f32)
    cp_excl = np.concatenate([np.ones((1,), f32), cp[:-1]])
    unorder = (1.0 - sorted_u) * cp_excl
    W_alloc = np.zeros((N,), f32)
    W_alloc[order] = unorder
    W_alloc = W_alloc[:, None]

    nM = _l2norm(M, 1)
    W_lookup_w = _softmax0(nM @ _l2norm(k_write, 1).T * b_write)
    W_write_new = (write_gate * (alloc_gate * W_alloc
                                 + (1.0 - alloc_gate) * W_lookup_w)).astype(f32)
    M_new = (M * (1.0 - W_write_new @ erase) + W_write_new @ write_v).astype(f32)

    w = W_write_new[:, 0]
    s = (1.0 - w).astype(f32)
    r = W_read.astype(f32)
    return dict(out_v=out_v.astype(f32), k_read=k_read, b_read=b_read,
                read_modes=read_modes, M_new=M_new, w=w, s=s, r=r,
                W_read_out=W_read_out, W_prec=W_prec)


def _make_stack(pr):
    """[N, 72] f32 stack S = [s*r, r, M*s] and per-column scales."""
    w, s, r, M = pr["w"], pr["s"], pr["r"], pr["M_new"]
    stack = np.concatenate(
        [s[:, None] * r, r, M * s[:, None]], axis=1).astype(np.float32)
    sy = np.empty(H0, np.float32)
    for cols in (slice(0, 8), slice(8, H0)):
        amax = float(np.abs(stack[:, cols]).max())
        e = np.clip(np.floor(np.log2(160.0 / max(amax, 1e-30))), -40.0, 40.0)
        sy[cols] = 2.0 ** e
    return stack, sy


def _host_post_q(pr, U, diagL, L):
    """Assemble y from U = S^T @ L [72, N] (f32, unscaled)."""
    w, s, r = pr["w"], pr["s"], pr["r"]
    p = pr["W_prec"][:, 0]
    M = pr["M_new"]
    Ld_new = ((1.0 - 2.0 * w) * diagL + w * p).astype(np.float32)

    B1 = U[0:4].T                       # L^T (s*r)   [N,4]
    B2 = U[4:8].T                       # L^T r       [N,4]
    V = U[8:H0]                         # (M*s)^T L   [64,N]
    wr = (w[:, None] * r).astype(np.float32)
    G1 = V @ r                          # (M*s)^T L r        [64,4]
    # G2 = M^T L (w*r) = V(w*r) + (M*w)^T L (w*r); the second term is
    # O(|w*r|^2)-tiny and dominated by the largest-w rows — recover it
    # exactly for the top-K rows from host-resident L.
    G2 = V @ wr
    idx = np.argpartition(w, N - KTOP)[N - KTOP:]
    q = L[idx, :].astype(np.float32) @ wr
    G2 = G2 + (M[idx] * w[idx, None]).T @ q

    pr_r = p @ r
    wr_r = w @ r
    Mld_r = (M * Ld_new[:, None]).T @ r
    Q_fwd = G1 - G2 + np.outer(M.T @ w, pr_r) - Mld_r
    Q_bwd = (M.T @ B1) - ((M * w[:, None]).T @ B2) \
        + np.outer(M.T @ p, wr_r) - Mld_r

    W_lookup_r = _softmax0(_l2norm(M, 1) @ _l2norm(pr["k_read"], 1).T
                           * pr["b_read"])
    Q_look = M.T @ W_lookup_r
    modes = pr["read_modes"]
    Q = modes[0][None, :] * Q_bwd + modes[1][None, :] * Q_look \
        + modes[2][None, :] * Q_fwd                     # [64, 4]

    read_v_new = Q.T                                    # [4, 64]
    y = pr["out_v"] + read_v_new.reshape(1, R * Wd) @ pr["W_read_out"]
    return y.astype(np.float32)


def _build_bass():
    """Raw-Bass SPMD kernel: U = S^T @ L, single row-layout fp8 pass.

    Inputs per core (host pre-swizzled):
      X  [NSLAB*128, 4096] fp8: slab s at rows [128s : 128s+128], SBUF
         partition p byte 512t+c = shard element [row 128t+p, col
         512s+c] of the L row shard.
      S  [128, RT*STK] fp8: partition p, element STK*t+j = stack column
         j of shard row 128t+p (j<72: [s*r, r, M*s]; 72:80 zero pad so
         the DoubleRow middle-dim stride is 16B-aligned).
      SC [128, 1] f32: per-U-row output scales (rows 0:72).
    One slab = one PSUM accumulation group of 4 fp8 DoubleRow matmuls
    (start->stop full bank lifecycle) rotating over the 8 banks. ACT
    copies even banks, DVE odd banks (they may touch PSUM concurrently
    on different banks), each scaling by SC and casting to fp8 into
    obx. Outputs drain in chunks on the sync HWDGE queue after all
    input dma_starts, overlapping the tail of the input stream; the
    last two slabs ship as quarter-DMAs (one per DoubleRow matmul's
    stripe) so only the final matmul trails the last transfer.
    """
    import concourse.bass as bass
    import concourse.mybir as mybir
    from contextlib import ExitStack

    f32 = mybir.dt.float32
    f8 = mybir.dt.float8e4
    nc = bass.Bass()
    X = nc.dram_tensor("X", [NSLAB * 128, 4096], f8, kind="ExternalInput")
    S = nc.dram_tensor("S", [128, RT * STK], f8, kind="ExternalInput")
    SC = nc.dram_tensor("SC", [128, 1], f32, kind="ExternalInput")
    U = nc.dram_tensor("U", [H0, N], f8, kind="ExternalOutput")

    # input transfer plan: coarse while the PE has slack, quarter-slab
    # at the very end so only the final matmul trails the stream. Each
    # transfer gets its own semaphore: a threshold on a shared counting
    # sem can be reached by a subset of fast SDMA engines running ahead
    # on later transfers, so per-transfer sems are the only safe signal.
    XFER_BYTES = [4 * 4096, 4 * 4096, 3 * 4096, 2 * 4096, 4096, 4096,
                  1024, 1024, 1024, 1024]
    XSPANS = []   # (byte_start, nbytes) within the [16*4096] column space
    off = 0
    for nb in XFER_BYTES:
        XSPANS.append((off, nb))
        off += nb
    assert off == NSLAB * 4096

    with ExitStack() as es:
        xb = es.enter_context(nc.sbuf_tensor("xb", [128, NB * 4096], f8))
        sb = es.enter_context(nc.sbuf_tensor("sb", [128, RT * STK], f8))
        scb = es.enter_context(nc.sbuf_tensor("scb", [128, 1], f32))
        obx = es.enter_context(nc.sbuf_tensor("obx", [H0, NSLAB * 512], f8))
        ps = [es.enter_context(nc.psum_tensor(f"ps{b}", [H0, 512], f32))
              for b in range(8)]
        sem_s = es.enter_context(nc.semaphore(name="sem_s"))
        tsem = [es.enter_context(nc.semaphore(name=f"t{i}"))
                for i in range(len(XSPANS))]
        pe_sem = es.enter_context(nc.semaphore(name="pe_sem"))
        act_c = es.enter_context(nc.semaphore(name="act_c"))
        dve_c = es.enter_context(nc.semaphore(name="dve_c"))
        out_sem = es.enter_context(nc.semaphore(name="out_sem"))
        blk = es.enter_context(nc.Block())

        # X DRAM bytes are laid out slab-major: column-space byte
        # 4096*s + c of the SBUF view = X[128*s + p, c]
        X3 = X.rearrange("(s p) c -> p s c", p=128)          # [128,16,4096]
        Xq = X.rearrange("(s p) (q c) -> p s q c", p=128, c=1024)
        xb3 = xb.rearrange("p (s c) -> p s c", c=4096)

        @blk.scalar
        def _(scalar):
            # stack + scale loads on the ACT HWDGE queue (parallel with
            # the X stream on the sync queue), then even-bank copies.
            # Both inc sem_s: 32 (= 16 engines x 2 transfers, per-engine
            # FIFO) implies both fully landed.
            scalar.dma_start(out=sb[:, :], in_=S[:, :]).then_inc(sem_s, 16)
            scalar.dma_start(out=scb[:, :], in_=SC[:, :]).then_inc(sem_s, 16)
            scalar.wait_ge(sem_s, 32)
            for g in range(0, NSLAB, 2):
                scalar.wait_ge(pe_sem, g + 1)
                scalar.mul(obx[:, 512 * g:512 * (g + 1)],
                           ps[g % 8][:, :],
                           scb[:H0, 0:1]).then_inc(act_c, 1)

        @blk.sync
        def _(sync):
            for i, (b0, nb) in enumerate(XSPANS):
                if nb % 4096 == 0:
                    s0, s1 = b0 // 4096, (b0 + nb) // 4096
                    sync.dma_start(
                        out=xb3[:, s0:s1, :],
                        in_=X3[:, s0:s1, :],
                    ).then_inc(tsem[i], 16)
                else:
                    s0, qq = b0 // 4096, (b0 % 4096) // 1024
                    sync.dma_start(
                        out=xb[:, b0:b0 + nb],
                        in_=Xq[:, s0:s0 + 1, qq:qq + 1, :],
                    ).then_inc(tsem[i], 16)
            # outputs behind all queued inputs; chunks get finer toward
            # the end so the last transfers are small and fire early
            for s0, s1 in ((0, 8), (8, 13), (13, 15), (15, 16)):
                sync.wait_ge(act_c, (s1 + 1) // 2)
                sync.wait_ge(dve_c, s1 // 2)
                sync.dma_start(
                    out=U[:, 512 * s0:512 * s1],
                    in_=obx[:, 512 * s0:512 * s1],
                ).then_inc(out_sem, 16)

        @blk.tensor
        def _(tensor):
            sbr = sb.rearrange("p (t j) -> p t j", j=STK)
            xbr = xb.rearrange("p (q c) -> p q c", c=512)

            # HAM warmup: the PE clock sits at 1.2 GHz until ~3.4us of
            # sustained activity. Burn the DMA lead-in on garbage
            # matmuls (last slab's SBUF region — not written until
            # ~30us) into bank 7 (cleared by slab 7's start=True) so
            # the real stream runs at 2.4 GHz from the first slab.
            for i in range(6):
                tensor.matmul(
                    ps[7][:, :],
                    xbr[:, 120:122, 0:H0],
                    xbr[:, 120:122, :],
                    start=True, stop=True,
                    perf_mode=mybir.MatmulPerfMode.DoubleRow,
                )
            tensor.wait_ge(sem_s, 32)   # stationary resident

            def mk_mm(s, u):
                mm = tensor.matmul(
                    ps[s % 8][:, :],
                    sbr[:, 2 * u:2 * u + 2, 0:H0],
                    xbr[:, 8 * s + 2 * u:8 * s + 2 * u + 2, :],
                    start=(u == 0), stop=(u == RT // 2 - 1),
                    perf_mode=mybir.MatmulPerfMode.DoubleRow,
                )
                if u == RT // 2 - 1:
                    mm.then_inc(pe_sem, 1)

            # per-slab gating: index of the transfer whose completion
            # covers byte (4096*s + 1024*(2u+2) - 1) of the column space
            def gate(s, u):
                need = 4096 * s + 1024 * (u + 1)
                for i, (b0, nb) in enumerate(XSPANS):
                    if b0 + nb >= need:
                        return i
                raise AssertionError

            last_gate = -1
            for s in range(NSLAB):
                if s >= 8:   # bank reuse: copier drained group s-8
                    if (s - 8) % 2 == 0:
                        tensor.wait_ge(act_c, (s - 8) // 2 + 1)
                    else:
                        tensor.wait_ge(dve_c, (s - 8) // 2 + 1)
                for u in range(RT // 2):
                    g = gate(s, u)
                    if g > last_gate:
                        tensor.wait_ge(tsem[g], 16)
                        last_gate = g
                    mk_mm(s, u)

        @blk.vector
        def _(vector):
            vector.wait_ge(sem_s, 32)
            for g in range(1, NSLAB, 2):    # odd banks on DVE
                vector.wait_ge(pe_sem, g + 1)
                vector.tensor_scalar_mul(
                    obx[:, 512 * g:512 * (g + 1)],
                    ps[g % 8][:, :],
                    scb[:H0, 0:1],
                ).then_inc(dve_c, 1)

    return nc


def _get_bass():
    if "nc" not in _cached:
        _cached["nc"] = _build_bass()
    return _cached["nc"]


def _swizzle_shard(S8):
    """[1024, 8192] fp8 shard -> [16*128, 4096] slab-major layout."""
    # want A[cc, p, t, c] = S8[128t + p, 512cc + c]
    a = S8.reshape(RT, 128, NSLAB, 512).transpose(2, 1, 0, 3)
    return np.ascontiguousarray(a).reshape(NSLAB * 128, RT * 512)


def _prep_core_inputs(L, stack, sy):
    """Quantize L (scaled) + stack to fp8, build per-core input maps.

    Also computes per-U-row power-of-2 output scales from the TRUE bound
    |U[j,:]| <= sum_i |S_q[i,j]| * max|L_q| (x1.14 quantization margin),
    so the on-device fp8 cast of scaled PSUM rows can never overflow.
    """
    L8 = (L * np.float32(SCALE)).astype(F8)
    scaled_f = stack * sy[None, :]
    scaled = scaled_f.astype(F8)
    st8 = np.zeros((N, STK), F8)
    st8[:, :H0] = scaled

    lmax = float(np.abs(L).max()) * SCALE
    bound = np.abs(scaled_f).sum(axis=0) * lmax * 1.14          # [72]
    e = np.clip(np.floor(np.log2(240.0 / np.maximum(bound, 1e-30))),
                -40.0, 40.0)
    sc = (2.0 ** e).astype(np.float32)                          # [72]
    SCm = np.zeros((128, 1), np.float32)
    SCm[:H0, 0] = sc

    in_maps = []
    for cid in range(NCORES):
        r0 = cid * SH
        Sh = np.ascontiguousarray(
            st8[r0:r0 + SH].reshape(RT, 128, STK).transpose(1, 0, 2)
            .reshape(128, RT * STK))
        in_maps.append({
            "X": _swizzle_shard(L8[r0:r0 + SH, :]),
            "S": Sh,
            "SC": SCm,
        })
    return in_maps, sc


def run_device(L, stack, sy, trace=False):
    """Returns (U [72, N] f32 unscaled, res) computed on 8 NeuronCores."""
    from concourse.bass_utils import run_bass_kernel_spmd
    nc = _get_bass()
    in_maps, sc = _prep_core_inputs(L, stack, sy)
    res = run_bass_kernel_spmd(nc, in_maps, core_ids=list(range(NCORES)),
                               trace=trace)
    U = np.zeros((H0, N), np.float64)
    for out in res.results:
        U += out["U"].astype(np.float64)
    U /= np.float64(SCALE)
    U /= (sy * sc)[:, None].astype(np.float64)
    return U.astype(np.float32), res


def kernel(**inputs):
    inputs = {k: np.asarray(v) for k, v in inputs.items()}
    L = inputs["L"]
    pr = _host_pre(**inputs)
    stack, sy = _make_stack(pr)
    U, _ = run_device(L, stack, sy)
    return _host_post_q(pr, U, np.diagonal(L).copy(), L)


if __name__ == "__main__":
    import reference
    inputs = reference.setup_inputs()
    y = kernel(**{k: np.asarray(v) for k, v in inputs.items()})
    print("y[0,:5] =", y[0, :5])


# revision 21
# speedup vs baseline: 1.2296x; 1.0023x over previous
"""DNC scatter_memory kernel for 8 Trainium2 NeuronCores.

Strategy: the only heavy tensor is L [8192,8192] (256MB f32). The
returned y depends on L only through a handful of row-contractions of
L, so the device computes a single fused product

  U = S^T @ L   with  S = [s*r, r, M*s]   [8192, 72]

row-sharded over 8 cores (U partials summed on host). Host assembles:

  B1 = L^T(s*r) = U[0:4].T,  B2 = L^T r = U[4:8].T,  V = (M*s)^T L
  Q_fwd = V r - (V(w*r) + Delta) + rank-1/diag fixups
  Q_bwd = M^T B1 - (M*w)^T B2 + rank-1/diag fixups

where Delta = (M*w)^T L (w*r) is O(|w*r|^2)-tiny; the host recovers it
near-exactly from the top-K rows of w (it owns L), so the device needs
only ONE 72-column stationary pass over L instead of the 136-column /
two-pass variant — PE streaming time halves and drops below the HBM
roofline for reading L once.

L is read exactly ONCE per element, as fp8_e4m3 (scaled 2^13; ~1e-4
end-to-end error vs the 2e-2 budget, since every output is an average
of 8192 independently-rounded terms). S is fp8 with per-column-group
power-of-2 scales chosen at runtime; U returns as fp8_e4m3, scaled per
row on-device (tensor_scalar_mul) with host-computed true-bound
power-of-2 scales so the cast can never overflow.

Per core: the 1024-row shard of L*2^13 (8MB fp8) is pre-swizzled on the
host into 16 contiguous 512KB "slabs" [128 part, 8 row-tiles x 512
cols]. One slab = one PSUM-bank accumulation group of 4 fp8-DoubleRow
matmuls (256-row contraction each, operand layout [p, 2, free]). Slabs
rotate through the 8 PSUM banks; ACT copies even banks, DVE odd banks,
both downcasting to fp8 into the obx staging buffer; outputs drain in
overlapping chunks on the sync HWDGE queue behind all input dma_starts
(SEQ-blocking waits can't stall queued input transfers). The last two
slabs ship as quarter-DMAs so only the final matmul trails the last
transfer.

Host side (LSTM controller, interface partition, usage/allocation sort,
content softmaxes, M update, Q assembly, final projection) is tiny.
"""
import numpy as np
import ml_dtypes

N, Wd, R, OUT, IN = 8192, 64, 4, 256, 256
NCORES = 8
SH = N // NCORES          # 1024 rows per core
RT = SH // 128            # 8 row tiles per shard
NSLAB = N // 512          # 16 slabs: one per 512-col chunk
NB = 16                   # slab ring depth (all slabs SBUF-resident)
STK = 80                  # stack row stride (72 cols + pad; %16 for DR)
H0 = 72                   # stationary width: [s*r, r, M*s]
SCALE = 8192.0            # L quantization scale (2^13)
KTOP = 128                # host-side Delta correction rows

F8 = ml_dtypes.float8_e4m3

_cached = {}


def _sig(x):
    return 1.0 / (1.0 + np.exp(-x))


def _softplus(x):
    return np.log1p(np.exp(-np.abs(x))) + np.maximum(x, 0.0)


def _softmax0(x):
    m = x.max(axis=0, keepdims=True)
    e = np.exp(x - m)
    return e / e.sum(axis=0, keepdims=True)


def _l2norm(a, axis):
    return a / np.sqrt(np.maximum((a * a).sum(axis=axis, keepdims=True), 1e-12))


def _host_pre(x, dense_k, dense_b, lstm_k, lstm_r, lstm_b, W_output, W_interface,
              W_read_out, h, c, M, usage, L, W_prec, W_read, W_write, read_v):
    f32 = np.float32
    xd = x.reshape(1, -1) @ dense_k + dense_b
    seq = np.concatenate([xd, read_v], axis=0)
    for t in range(seq.shape[0]):
        z = seq[t][None, :] @ lstm_k + h @ lstm_r + lstm_b
        zi, zf, zg, zo = np.split(z, 4, axis=1)
        c = _sig(zf) * c + _sig(zi) * np.tanh(zg)
        h = _sig(zo) * np.tanh(c)
    out_v = h @ W_output
    iface = (h @ W_interface)[0]

    sizes = [R * Wd, R, Wd, 1, Wd, Wd, R, 1, 1, 3 * R]
    offs = np.cumsum([0] + sizes)
    p = [iface[offs[i]:offs[i + 1]] for i in range(10)]
    k_read = p[0].reshape(R, Wd)
    b_read = 1.0 + _softplus(p[1])[None, :]
    k_write = p[2][None, :]
    b_write = 1.0 + _softplus(p[3])[None, :]
    erase = _sig(p[4])[None, :]
    write_v = p[5][None, :]
    free_gates = _sig(p[6])[None, :]
    alloc_gate = _sig(p[7][0])
    write_gate = _sig(p[8][0])
    read_modes = _softmax0(p[9].reshape(3, R))

    retention = np.prod(1.0 - free_gates * W_read, axis=1, keepdims=True).astype(f32)
    usage = ((usage + W_write - usage * W_write) * retention).astype(f32)
    u_flat = usage[:, 0]
    order = np.argsort(u_flat, kind="stable")
    sorted_u = u_flat[order]
    cp = np.cumprod(sorted_u).astype(f32)
    cp_excl = np.concatenate([np.ones((1,), f32), cp[:-1]])
    unorder = (1.0 - sorted_u) * cp_excl
    W_alloc = np.zeros((N,), f32)
    W_alloc[order] = unorder
    W_alloc = W_alloc[:, None]

    nM = _l2norm(M, 1)
    W_lookup_w = _softmax0(nM @ _l2norm(k_write, 1).T * b_write)
    W_write_new = (write_gate * (alloc_gate * W_alloc
                                 + (1.0 - alloc_gate) * W_lookup_w)).astype(f32)
    M_new = (M * (1.0 - W_write_new @ erase) + W_write_new @ write_v).astype(f32)

    w = W_write_new[:, 0]
    s = (1.0 - w).astype(f32)
    r = W_read.astype(f32)
    return dict(out_v=out_v.astype(f32), k_read=k_read, b_read=b_read,
                read_modes=read_modes, M_new=M_new, w=w, s=s, r=r,
                W_read_out=W_read_out, W_prec=W_prec)


def _make_stack(pr):
    """[N, 72] f32 stack S = [s*r, r, M*s] and per-column scales."""
    w, s, r, M = pr["w"], pr["s"], pr["r"], pr["M_new"]
    stack = np.concatenate(
        [s[:, None] * r, r, M * s[:, None]], axis=1).astype(np.float32)
    sy = np.empty(H0, np.float32)
    for cols in (slice(0, 8), slice(8, H0)):
        amax = float(np.abs(stack[:, cols]).max())
        e = np.clip(np.floor(np.log2(160.0 / max(amax, 1e-30))), -40.0, 40.0)
        sy[cols] = 2.0 ** e
    return stack, sy


def _host_post_q(pr, U, diagL, L):
    """Assemble y from U = S^T @ L [72, N] (f32, unscaled)."""
    w, s, r = pr["w"], pr["s"], pr["r"]
    p = pr["W_prec"][:, 0]
    M = pr["M_new"]
    Ld_new = ((1.0 - 2.0 * w) * diagL + w * p).astype(np.float32)

    B1 = U[0:4].T                       # L^T (s*r)   [N,4]
    B2 = U[4:8].T                       # L^T r       [N,4]
    V = U[8:H0]                         # (M*s)^T L   [64,N]
    wr = (w[:, None] * r).astype(np.float32)
    G1 = V @ r                          # (M*s)^T L r        [64,4]
    # G2 = M^T L (w*r) = V(w*r) + (M*w)^T L (w*r); the second term is
    # O(|w*r|^2)-tiny and dominated by the largest-w rows — recover it
    # exactly for the top-K rows from host-resident L.
    G2 = V @ wr
    idx = np.argpartition(w, N - KTOP)[N - KTOP:]
    q = L[idx, :].astype(np.float32) @ wr
    G2 = G2 + (M[idx] * w[idx, None]).T @ q

    pr_r = p @ r
    wr_r = w @ r
    Mld_r = (M * Ld_new[:, None]).T @ r
    Q_fwd = G1 - G2 + np.outer(M.T @ w, pr_r) - Mld_r
    Q_bwd = (M.T @ B1) - ((M * w[:, None]).T @ B2) \
        + np.outer(M.T @ p, wr_r) - Mld_r

    W_lookup_r = _softmax0(_l2norm(M, 1) @ _l2norm(pr["k_read"], 1).T
                           * pr["b_read"])
    Q_look = M.T @ W_lookup_r
    modes = pr["read_modes"]
    Q = modes[0][None, :] * Q_bwd + modes[1][None, :] * Q_look \
        + modes[2][None, :] * Q_fwd                     # [64, 4]

    read_v_new = Q.T                                    # [4, 64]
    y = pr["out_v"] + read_v_new.reshape(1, R * Wd) @ pr["W_read_out"]
    return y.astype(np.float32)


def _build_bass():
    """Raw-Bass SPMD kernel: U = S^T @ L, single row-layout fp8 pass.

    Inputs per core (host pre-swizzled):
      X  [NSLAB*128, 4096] fp8: slab s at rows [128s : 128s+128], SBUF
         partition p byte 512t+c = shard element [row 128t+p, col
         512s+c] of the L row shard.
      S  [128, RT*STK] fp8: partition p, element STK*t+j = stack column
         j of shard row 128t+p (j<72: [s*r, r, M*s]; 72:80 zero pad so
         the DoubleRow middle-dim stride is 16B-aligned).
      SC [128, 1] f32: per-U-row output scales (rows 0:72).
    One slab = one PSUM accumulation group of 4 fp8 DoubleRow matmuls
    (start->stop full bank lifecycle) rotating over the 8 banks. ACT
    copies even banks, DVE odd banks (they may touch PSUM concurrently
    on different banks), each scaling by SC and casting to fp8 into
    obx. Outputs drain in chunks on the sync HWDGE queue after all
    input dma_starts, overlapping the tail of the input stream; the
    last two slabs ship as quarter-DMAs (one per DoubleRow matmul's
    stripe) so only the final matmul trails the last transfer.
    """
    import concourse.bass as bass
    import concourse.mybir as mybir
    from contextlib import ExitStack

    f32 = mybir.dt.float32
    f8 = mybir.dt.float8e4
    nc = bass.Bass()
    X = nc.dram_tensor("X", [NSLAB * 128, 4096], f8, kind="ExternalInput")
    S = nc.dram_tensor("S", [128, RT * STK], f8, kind="ExternalInput")
    SC = nc.dram_tensor("SC", [128, 1], f32, kind="ExternalInput")
    U = nc.dram_tensor("U", [H0, N], f8, kind="ExternalOutput")

    # input transfer plan: coarse while the PE has slack, quarter-slab
    # at the very end so only the final matmul trails the stream. Each
    # transfer gets its own semaphore: a threshold on a shared counting
    # sem can be reached by a subset of fast SDMA engines running ahead
    # on later transfers, so per-transfer sems are the only safe signal.
    XFER_BYTES = [4096, 4096, 4096, 4096, 3 * 4096, 3 * 4096, 3 * 4096,
                  4096, 4096, 1024, 1024, 1024, 1024]
    XSPANS = []   # (byte_start, nbytes) within the [16*4096] column space
    off = 0
    for nb in XFER_BYTES:
        XSPANS.append((off, nb))
        off += nb
    assert off == NSLAB * 4096

    with ExitStack() as es:
        xb = es.enter_context(nc.sbuf_tensor("xb", [128, NB * 4096], f8))
        sb = es.enter_context(nc.sbuf_tensor("sb", [128, RT * STK], f8))
        scb = es.enter_context(nc.sbuf_tensor("scb", [128, 1], f32))
        obx = es.enter_context(nc.sbuf_tensor("obx", [H0, NSLAB * 512], f8))
        ps = [es.enter_context(nc.psum_tensor(f"ps{b}", [H0, 512], f32))
              for b in range(8)]
        sem_s = es.enter_context(nc.semaphore(name="sem_s"))
        tsem = [es.enter_context(nc.semaphore(name=f"t{i}"))
                for i in range(len(XSPANS))]
        pe_sem = es.enter_context(nc.semaphore(name="pe_sem"))
        act_c = es.enter_context(nc.semaphore(name="act_c"))
        dve_c = es.enter_context(nc.semaphore(name="dve_c"))
        out_sem = es.enter_context(nc.semaphore(name="out_sem"))
        blk = es.enter_context(nc.Block())

        # X DRAM bytes are laid out slab-major: column-space byte
        # 4096*s + c of the SBUF view = X[128*s + p, c]
        X3 = X.rearrange("(s p) c -> p s c", p=128)          # [128,16,4096]
        Xq = X.rearrange("(s p) (q c) -> p s q c", p=128, c=1024)
        xb3 = xb.rearrange("p (s c) -> p s c", c=4096)

        @blk.scalar
        def _(scalar):
            # even-bank copies, plus the right half of the final bank
            scalar.wait_ge(sem_s, 32)
            for g in range(0, NSLAB, 2):
                scalar.wait_ge(pe_sem, g + 1)
                scalar.mul(obx[:, 512 * g:512 * (g + 1)],
                           ps[g % 8][:, :],
                           scb[:H0, 0:1]).then_inc(act_c, 1)


        @blk.sync
        def _(sync):
            # stack + scale loads first (tiny; the PE warmup covers the
            # delay they add to the X stream), then the X transfers.
            # Both inc sem_s: 32 (= 16 engines x 2 transfers, per-engine
            # FIFO) implies both fully landed.
            sync.dma_start(out=sb[:, :], in_=S[:, :]).then_inc(sem_s, 16)
            sync.dma_start(out=scb[:, :], in_=SC[:, :]).then_inc(sem_s, 16)
            for i, (b0, nb) in enumerate(XSPANS):
                if nb % 4096 == 0:
                    s0, s1 = b0 // 4096, (b0 + nb) // 4096
                    sync.dma_start(
                        out=xb3[:, s0:s1, :],
                        in_=X3[:, s0:s1, :],
                    ).then_inc(tsem[i], 16)
                else:
                    s0, qq = b0 // 4096, (b0 % 4096) // 1024
                    sync.dma_start(
                        out=xb[:, b0:b0 + nb],
                        in_=Xq[:, s0:s0 + 1, qq:qq + 1, :],
                    ).then_inc(tsem[i], 16)
            # outputs behind all queued inputs; chunks get finer toward
            # the end so the last transfers are small and fire early.
            # (wait thresholds: copies of all groups < s1 done, with the
            # final bank split across ACT and DVE)
            for s0, s1, na, nd in ((0, 8, 4, 4), (8, 13, 7, 6),
                                   (13, 15, 8, 7), (15, 16, 8, 8)):
                sync.wait_ge(act_c, na)
                sync.wait_ge(dve_c, nd)
                sync.dma_start(
                    out=U[:, 512 * s0:512 * s1],
                    in_=obx[:, 512 * s0:512 * s1],
                ).then_inc(out_sem, 16)

        @blk.tensor
        def _(tensor):
            sbr = sb.rearrange("p (t j) -> p t j", j=STK)
            xbr = xb.rearrange("p (q c) -> p q c", c=512)

            # HAM warmup: the PE clock sits at 1.2 GHz until ~3.4us of
            # sustained activity. Burn the DMA lead-in on garbage
            # matmuls (last slab's SBUF region — not written until
            # ~30us) into bank 7 (cleared by slab 7's start=True) so
            # the real stream runs at 2.4 GHz from the first slab.
            for i in range(8):
                tensor.matmul(
                    ps[7][:, :],
                    xbr[:, 120:122, 0:H0],
                    xbr[:, 120:122, :],
                    start=True, stop=True,
                    perf_mode=mybir.MatmulPerfMode.DoubleRow,
                )
            tensor.wait_ge(sem_s, 32)   # stationary resident

            def mk_mm(s, u):
                mm = tensor.matmul(
                    ps[s % 8][:, :],
                    sbr[:, 2 * u:2 * u + 2, 0:H0],
                    xbr[:, 8 * s + 2 * u:8 * s + 2 * u + 2, :],
                    start=(u == 0), stop=(u == RT // 2 - 1),
                    perf_mode=mybir.MatmulPerfMode.DoubleRow,
                )
                if u == RT // 2 - 1:
                    mm.then_inc(pe_sem, 1)

            # per-slab gating: index of the transfer whose completion
            # covers byte (4096*s + 1024*(2u+2) - 1) of the column space
            def gate(s, u):
                need = 4096 * s + 1024 * (u + 1)
                for i, (b0, nb) in enumerate(XSPANS):
                    if b0 + nb >= need:
                        return i
                raise AssertionError

            last_gate = -1
            for s in range(NSLAB):
                if s >= 8:   # bank reuse: copier drained group s-8
                    if (s - 8) % 2 == 0:
                        tensor.wait_ge(act_c, (s - 8) // 2 + 1)
                    else:
                        tensor.wait_ge(dve_c, (s - 8) // 2 + 1)
                for u in range(RT // 2):
                    g = gate(s, u)
                    if g > last_gate:
                        tensor.wait_ge(tsem[g], 16)
                        last_gate = g
                    mk_mm(s, u)

        @blk.vector
        def _(vector):
            # odd-bank copies except the final bank, whose copy is
            # split: DVE takes the left half, ACT the right, so the
            # last copy latency halves on the critical tail
            vector.wait_ge(sem_s, 32)
            for g in range(1, NSLAB, 2):
                vector.wait_ge(pe_sem, g + 1)
                vector.tensor_scalar_mul(
                    obx[:, 512 * g:512 * (g + 1)],
                    ps[g % 8][:, :],
                    scb[:H0, 0:1],
                ).then_inc(dve_c, 1)

    return nc


def _get_bass():
    if "nc" not in _cached:
        _cached["nc"] = _build_bass()
    return _cached["nc"]


def _swizzle_shard(S8):
    """[1024, 8192] fp8 shard -> [16*128, 4096] slab-major layout."""
    # want A[cc, p, t, c] = S8[128t + p, 512cc + c]
    a = S8.reshape(RT, 128, NSLAB, 512).transpose(2, 1, 0, 3)
    return np.ascontiguousarray(a).reshape(NSLAB * 128, RT * 512)


def _prep_core_inputs(L, stack, sy):
    """Quantize L (scaled) + stack to fp8, build per-core input maps.

    Also computes per-U-row power-of-2 output scales from the TRUE bound
    |U[j,:]| <= sum_i |S_q[i,j]| * max|L_q| (x1.14 quantization margin),
    so the on-device fp8 cast of scaled PSUM rows can never overflow.
    """
    L8 = (L * np.float32(SCALE)).astype(F8)
    scaled_f = stack * sy[None, :]
    scaled = scaled_f.astype(F8)
    st8 = np.zeros((N, STK), F8)
    st8[:, :H0] = scaled

    lmax = float(np.abs(L).max()) * SCALE
    bound = np.abs(scaled_f).sum(axis=0) * lmax * 1.14          # [72]
    e = np.clip(np.floor(np.log2(240.0 / np.maximum(bound, 1e-30))),
                -40.0, 40.0)
    sc = (2.0 ** e).astype(np.float32)                          # [72]
    SCm = np.zeros((128, 1), np.float32)
    SCm[:H0, 0] = sc

    in_maps = []
    for cid in range(NCORES):
        r0 = cid * SH
        Sh = np.ascontiguousarray(
            st8[r0:r0 + SH].reshape(RT, 128, STK).transpose(1, 0, 2)
            .reshape(128, RT * STK))
        in_maps.append({
            "X": _swizzle_shard(L8[r0:r0 + SH, :]),
            "S": Sh,
            "SC": SCm,
        })
    return in_maps, sc


def run_device(L, stack, sy, trace=False):
    """Returns (U [72, N] f32 unscaled, res) computed on 8 NeuronCores."""
    from concourse.bass_utils import run_bass_kernel_spmd
    nc = _get_bass()
    in_maps, sc = _prep_core_inputs(L, stack, sy)
    res = run_bass_kernel_spmd(nc, in_maps, core_ids=list(range(NCORES)),
                               trace=trace)
    U = np.zeros((H0, N), np.float64)
    for out in res.results:
        U += out["U"].astype(np.float64)
    U /= np.float64(SCALE)
    U /= (sy * sc)[:, None].astype(np.float64)
    return U.astype(np.float32), res


def kernel(**inputs):
    inputs = {k: np.asarray(v) for k, v in inputs.items()}
    L = inputs["L"]
    pr = _host_pre(**inputs)
    stack, sy = _make_stack(pr)
    U, _ = run_device(L, stack, sy)
    return _host_post_q(pr, U, np.diagonal(L).copy(), L)


if __name__ == "__main__":
    import reference
    inputs = reference.setup_inputs()
    y = kernel(**{k: np.asarray(v) for k, v in inputs.items()})
    print("y[0,:5] =", y[0, :5])
